# revision 42
# baseline (speedup 1.0000x reference)
"""Trainium2 Bass kernel for nn_AttentionHead (B=4, S=2048, D_IN=D_OUT=1024).

Sharding: 8 cores; core c handles batch b=c//2 and half the queries,
balanced for causal load: even cores q in [0,512)+[1536,2048), odd cores
q in [512,1536).  Each core computes the full K^T / V projections for its
batch (duplicated within the core pair) plus causal attention for its own
queries, organized as two uniform 512-query phase slots with K_slot=(8,16)
key-tiles.  Causal masking AND the slot padding are data-driven via
host-sent per-partition thresholds (mask = iota >= thr applied to exp(S)),
so all 8 cores run one identical SPMD program.

All matmuls use bf16 operands with fp32 PSUM accumulation (full-rate
TensorE at free-dim 512, and LDWEIGHTS rides fast-weight-load so it
hides under the matmuls; end-to-end rel err ~1e-3).  Everything is
computed transposed so no on-chip transposes
are ever needed:
  stage A: K^T[e,k] = Wk-tiles.T @ Xk^T      (host pre-transposes X into
           SBUF-ready [d_p, d_o, s] blocks; d-outer loop so the first
           matmul needs only one 256KB strip)
  stage C: Q^T[e,q] = Wq-tiles.T @ Xq^T      (Wq rows overwrite the wk
           tile in place; range-based WAR keeps it pipelined)
  stage B: V[k,e]  = Xv^T-tiles.T @ Wv       (staged to DRAM in an
           [k_p, et, k_o, e] layout so stage-D slab reads are contiguous)
  stage D per slot: S^T[k,q] = KT-tiles.T @ Q^T, exp+mask on S^T,
           den = ones.T @ expS (denominator replicated on all partitions),
           O^T[e,q] = V-slab-tiles.T @ expS^T, scaled by 1/den.
Output is O^T per core; the host reassembles [B,S,D].

Perf notes: ~296us HW time on 8 cores (TensorE ~88% busy, matmul p50
230ns at N=512).  DMA queues: bulk loads ride HWDGE (nc.sync), stores +
small loads ride SWDGE (nc.gpsimd) so PE load-waits never sit behind
result-dependent stores; walrus accepts only ONE sync-wait per
instruction, so _split_multi_waits() splits extras onto wait-only NoOps.
"""
import sys
import types

sys.path.insert(0, "/opt/trn_rl_repo")


def _install_ntff_hook():
    try:
        import antenv
    except ImportError:
        return

    if "antenv.axon_hooks" in sys.modules:
        return
    mod = types.ModuleType("antenv.axon_hooks")
    _h = [None]
    mod.set_axon_ntff_profile_hook = lambda h: _h.__setitem__(0, h)
    mod.get_axon_ntff_profile_hook = lambda: _h[0]
    sys.modules["antenv.axon_hooks"] = mod
    antenv.axon_hooks = mod
    try:
        from trn_agent_boot.trn_boot import _ntff_profile_via_ctypes

        mod.set_axon_ntff_profile_hook(
            _ntff_profile_via_ctypes("/opt/axon/libaxon_pjrt.so"))
    except Exception:
        pass


_install_ntff_hook()


import numpy as np
import concourse.bass as bass
import concourse.tile as tile
from concourse import mybir
from concourse.bass_utils import run_bass_kernel_spmd

P = 128
B, S, D = 4, 2048, 1024
N = 512                      # matmul moving free dim / queries per slot
NCORES = 8
K_SLOTS = (8, 16)            # k-tiles per phase slot (uniform across cores)
Q0S = {0: (0, 1536), 1: (512, 1024)}   # slot query starts per core parity
SCALE = float(1.0 / np.sqrt(np.float32(2048)))

f32 = mybir.dt.float32
bf16 = mybir.dt.bfloat16
fp8 = mybir.dt.float8e4
DR = mybir.MatmulPerfMode.DoubleRow
EXP = mybir.ActivationFunctionType.Exp
MULT = mybir.AluOpType.mult


def _split_multi_waits(nc):
    """Walrus allows one sync-wait per instruction; split extras onto
    wait-only NoOps inserted right before the offending instruction."""
    for f in nc.m.functions:
        for bb in f.blocks:
            insts = bb.instructions
            i = 0
            while i < len(insts):
                ins = insts[i]
                si = getattr(ins, "sync_info", None)
                if si and si.on_wait and len(si.on_wait) > 1:
                    waits = list(si.on_wait)
                    for j, w in enumerate(waits[:-1]):
                        nop = mybir.InstNoOp(
                            name=f"{ins.name}-waitsplit-{j}",
                            sync_info=mybir.SyncInfo(on_wait=[w], on_update=[]),
                            bass_nofuse=True,
                            engine=ins.engine, ins=[], outs=[])
                        insts.insert(i + j, nop)
                    i += len(waits) - 1
                    ins.sync_info = mybir.SyncInfo(
                        on_wait=[waits[-1]], on_update=list(si.on_update))
                i += 1


def build():
    nc = bass.Bass()
    # all host-side tensors are pre-arranged into SBUF layout [dp, do, cols]
    wq = nc.dram_tensor("wq", [P, 8, 8, P], fp8, kind="ExternalInput")
    wk = nc.dram_tensor("wk", [P, 8, D], fp8, kind="ExternalInput")
    wv = nc.dram_tensor("wv", [P, 8, D], bf16, kind="ExternalInput")
    wv8 = nc.dram_tensor("wv8", [P, 8, D], fp8, kind="ExternalInput")
    xqt = nc.dram_tensor("xqt", [P, 8, 1024], fp8, kind="ExternalInput")
    xkt = nc.dram_tensor("xkt", [P, 8, S], fp8, kind="ExternalInput")
    xvt = nc.dram_tensor("xvt", [P, 8, 1024], bf16, kind="ExternalInput")
    xvt8 = nc.dram_tensor("xvt8", [P, 8, 1024], fp8, kind="ExternalInput")
    thr = nc.dram_tensor("thr", [P, 2, 16], f32, kind="ExternalInput")
    iot = nc.dram_tensor("iota", [P, N], f32, kind="ExternalInput")
    out = nc.dram_tensor("out", [D, 1024], f32, kind="ExternalOutput")

    with tile.TileContext(nc) as tc:
        from contextlib import ExitStack
        with ExitStack() as ctx:
            kt_pool = ctx.enter_context(tc.tile_pool(name="ktp", bufs=1))
            xh_pool = ctx.enter_context(tc.tile_pool(name="xh", bufs=1))
            qt_pool = ctx.enter_context(tc.tile_pool(name="qtp", bufs=1))
            sm_pool = ctx.enter_context(tc.tile_pool(name="sm", bufs=1))
            psum = ctx.enter_context(
                tc.tile_pool(name="ps", bufs=8, space="PSUM"))
            dram = ctx.enter_context(
                tc.tile_pool(name="dram", bufs=1, space="DRAM"))

            v_dram = dram.tile([P, 8, 16, P], bf16)  # V: [k_p, et, k_o, e]
            v8_dram = dram.tile([P, 8, 16, P], fp8)  # fp8 copy for slot 1
            KT = kt_pool.tile([P, 8, S], fp8)        # K^T: [e_p, e_o, k]
            QT = qt_pool.tile([P, 8, 1024], fp8)     # Q^T: [e_p, e_o, q_loc]

            ones = sm_pool.tile([P, P], bf16)
            nc.gpsimd.memset(ones[:], 1.0)
            ones8 = sm_pool.tile([P, 2, P], fp8)
            nc.gpsimd.memset(ones8[:], 1.0)

            ET_GROUPS = ((0, 2), (2, 4), (4, 6), (6, 8))

            # warm up the PE HAM clock while the first input strips stream in
            wps = psum.tile([P, N], f32, tag="ps", name="warmps")
            for i in range(24):
                nc.tensor.matmul(wps[:, 0:P], ones[:], ones[:],
                                 start=(i == 0), stop=(i == 23))

            def copy_alt(i, dst, src):
                if i % 2 == 0:
                    nc.vector.tensor_copy(dst, src)
                else:
                    nc.scalar.copy(dst, src)

            # w_sb is overwritten in place three times (wk -> wq -> wv);
            # range-based tracking gives per-row WAR deps, so each overwrite
            # streams in while later rows are still being consumed.
            with tc.tile_pool(name="wres", bufs=1) as wres:
                w_sb = wres.tile([P, 8, D], fp8, tag="w")

                # ---- Stage A: K^T[e,k] = sum_d Wk-tiles.T @ Xk^T[d,k] ----
                # loads ride 3 queues in d-pair strips matching DR t-pairs,
                # so descriptor generation (~0.7us/desc/queue) never starves
                # the PE.
                QS = (nc.sync, nc.scalar, nc.gpsimd)
                # double-buffered xk halves: every load below is dep-free, so
                # the queues stream back-to-back with no head-of-line WAR
                # blocking.
                xk_hs = [xh_pool.tile([P, 8, 1024], fp8, tag=f"xk{h}",
                                      name=f"xk{h}") for h in range(2)]
                for d in range(8):
                    QS[d % 3].dma_start(w_sb[:, d, :], wk[:, d, :])
                    QS[(d + 1) % 3].dma_start(xk_hs[0][:, d, :],
                                              xkt[:, d, 0:1024])
                for d in range(8):
                    QS[(d + 2) % 3].dma_start(xk_hs[1][:, d, :],
                                              xkt[:, d, 1024:2048])
                for half in range(2):
                    xk_h = xk_hs[half]
                    for g0, g1 in ET_GROUPS:
                        pss = {}
                        for et in range(g0, g1):
                            for kc in range(2):
                                pss[(et, kc)] = psum.tile(
                                    [P, N], f32, tag="ps",
                                    name=f"psa{half}_{et}_{kc}")
                        for t in range(4):
                            for et in range(g0, g1):
                                lhs = w_sb[:, 2 * t:2 * t + 2,
                                           et * P:(et + 1) * P]
                                for kc in range(2):
                                    nc.tensor.matmul(
                                        pss[(et, kc)][:], lhs,
                                        xk_h[:, 2 * t:2 * t + 2,
                                             kc * N:(kc + 1) * N],
                                        start=(t == 0), stop=(t == 3),
                                        perf_mode=DR)
                        for i, et in enumerate(range(g0, g1)):
                            for kc in range(2):
                                col = half * 1024 + kc * N
                                copy_alt(i + kc, KT[:, et, col:col + N],
                                         pss[(et, kc)][:])

                # ---- Stage C: Q^T[e,q] = sum_d Wq-tiles.T @ Xq^T[d,q] ----
                # wq gets its own tile (no WAR on w_sb) so its loads stream
                # during stage A; wq_sb[:, et, d*P:(d+1)*P] holds
                # Wq[d*P:(d+1)*P, et*P:(et+1)*P]
                wq_sb = wres.tile([P, 8, D], fp8, tag="wq", name="wq_sb")
                for et in range(8):
                    QS[et % 3].dma_start(wq_sb[:, et, :], wq[:, et, :, :])
                with tc.tile_pool(name="xqs", bufs=8) as xq_pool:
                    xqhs = {}
                    for qc in range(2):
                        for t in range(4):
                            xqh = xq_pool.tile([P, 2, N], fp8, tag="xq",
                                               name=f"xq{qc}_{t}")
                            QS[(qc * 4 + t) % 3].dma_start(
                                xqh[:],
                                xqt[:, 2 * t:2 * t + 2, qc * N:(qc + 1) * N])
                            xqhs[(qc, t)] = xqh
                    # 4 PSUM banks per et-group so consecutive groups pipeline
                    for qc in range(2):
                        for eg in range(2):
                            ets = range(4 * eg, 4 * eg + 4)
                            psq = {et: psum.tile([P, N], f32, tag="ps",
                                                 name=f"psq{qc}_{et}")
                                   for et in ets}
                            for t in range(4):
                                for et in ets:
                                    lhs = wq_sb[
                                        :, et, 2 * t * P:(2 * t + 2) * P
                                    ].rearrange("p (two f) -> p two f", two=2)
                                    nc.tensor.matmul(
                                        psq[et][:], lhs, xqhs[(qc, t)][:],
                                        start=(t == 0), stop=(t == 3),
                                        perf_mode=DR)
                            for et in ets:
                                copy_alt(et, QT[:, et, qc * N:(qc + 1) * N],
                                         psq[et][:])

            # ---- Stage B: V[k,e] = sum_d Xv^T-tiles.T @ Wv[d,e] ----
            iota_sb = sm_pool.tile([P, N], f32)
            nc.sync.dma_start(iota_sb[:], iot[:])
            thr_sb = sm_pool.tile([P, 2, 16], f32)
            nc.sync.dma_start(thr_sb[:], thr[:])
            if True:  # keep indent level of the former wres scope
                with tc.tile_pool(name="vp", bufs=3) as v_pool, \
                        tc.tile_pool(name="wvp", bufs=1) as wv_pool:
                    w_sb = wv_pool.tile([P, 8, D], bf16, tag="wv")
                    wv8_sb = wv_pool.tile([P, 8, D], fp8, tag="wv8",
                                          name="wv8_sb")
                    for d in range(8):
                        QS[d % 3].dma_start(w_sb[:, d, :], wv[:, d, :])
                    # half 0 (keys 0..1023, feeds the earliest queries) stays
                    # bf16; half 1 (keys 1024..2047, only ever attended with
                    # n_eff >= 1024) runs fp8 DoubleRow.
                    xv_h0 = xh_pool.tile([P, 8, 1024], bf16, tag="xv0",
                                         name="xv0")
                    xv_h1 = xh_pool.tile([P, 8, 1024], fp8, tag="xv1",
                                         name="xv1")
                    for d in range(8):
                        QS[(d + 1) % 3].dma_start(xv_h0[:, d, :],
                                                  xvt[:, d, :])
                    for d in range(8):
                        QS[(d + 2) % 3].dma_start(xv_h1[:, d, :],
                                                  xvt8[:, d, :])
                        QS[d % 3].dma_start(wv8_sb[:, d, :], wv8[:, d, :])
                    for half in range(2):
                        for g0, g1 in ((0, 2), (2, 4), (4, 6), (6, 8)):
                            ps2 = {}
                            for ktl in range(g0, g1):
                                for ec in range(2):
                                    ps2[(ktl, ec)] = psum.tile(
                                        [P, N], f32, tag="ps",
                                        name=f"psb{half}_{ktl}_{ec}")
                            if half == 0:
                                for d in range(8):
                                    for ktl in range(g0, g1):
                                        lhs = xv_h0[:, d,
                                                    ktl * P:(ktl + 1) * P]
                                        for ec in range(2):
                                            nc.tensor.matmul(
                                                ps2[(ktl, ec)][:], lhs,
                                                w_sb[:, d,
                                                     ec * N:(ec + 1) * N],
                                                start=(d == 0), stop=(d == 7))
                            else:
                                for t in range(4):
                                    for ktl in range(g0, g1):
                                        lhs = xv_h1[:, 2 * t:2 * t + 2,
                                                    ktl * P:(ktl + 1) * P]
                                        for ec in range(2):
                                            nc.tensor.matmul(
                                                ps2[(ktl, ec)][:], lhs,
                                                wv8_sb[:, 2 * t:2 * t + 2,
                                                       ec * N:(ec + 1) * N],
                                                start=(t == 0), stop=(t == 3),
                                                perf_mode=DR)
                            for ktl in range(g0, g1):
                                ktg = half * 8 + ktl
                                for ec in range(2):
                                    vt8 = v_pool.tile([P, N], fp8,
                                                      tag="vst8", name="vt8")
                                    if half == 0:
                                        # slot 0 needs bf16 V; slot 1 reads
                                        # the fp8 copy (cast on idle DVE)
                                        vt = v_pool.tile([P, N], bf16,
                                                         tag="vst")
                                        nc.scalar.copy(vt[:],
                                                       ps2[(ktl, ec)][:])
                                        nc.vector.tensor_copy(vt8[:], vt[:])
                                        nc.scalar.dma_start(
                                            v_dram[:, ec * 4:(ec + 1) * 4,
                                                   ktg, :],
                                            vt[:].rearrange(
                                                "p (et e) -> p et e", e=P))
                                    else:
                                        nc.scalar.copy(vt8[:],
                                                       ps2[(ktl, ec)][:])
                                    nc.gpsimd.dma_start(
                                        v8_dram[:, ec * 4:(ec + 1) * 4,
                                                ktg, :],
                                        vt8[:].rearrange(
                                            "p (et e) -> p et e", e=P))

            # ---- Stage D: per phase slot: scores, softmax, O^T ----
            # masks precomputed on DVE (overlaps stage B); the per-kt apply
            # rides the otherwise-idle gpsimd engine.
            vin_pool = ctx.enter_context(tc.tile_pool(name="vin", bufs=6))
            out_pool = ctx.enter_context(tc.tile_pool(name="op", bufs=3))
            mk_pool = ctx.enter_context(tc.tile_pool(name="mk", bufs=1))
            rd_pool = ctx.enter_context(tc.tile_pool(name="rd", bufs=2))
            v8_pool = ctx.enter_context(tc.tile_pool(name="v8", bufs=3))
            masks = {}
            for s in range(2):
                mdt = bf16 if s == 0 else fp8
                for kt in range(K_SLOTS[s]):
                    if s == 1 and kt < 8:
                        continue
                    mk = mk_pool.tile([P, N], mdt, tag=f"mk{s}_{kt}",
                                      name=f"mk{s}_{kt}")
                    nc.vector.tensor_scalar(
                        out=mk[:], in0=iota_sb[:],
                        scalar1=thr_sb[:, s, kt:kt + 1], scalar2=None,
                        op0=mybir.AluOpType.is_ge)
                    masks[(s, kt)] = mk
            for s in range(2):
                K = K_SLOTS[s]
                # slot 0 holds the earliest queries -> bf16 attention weights
                # and V; slot 1 (n_eff >= 1024 keys) runs fp8 end-to-end.
                sdt = bf16 if s == 0 else fp8
                # scores^T -> exp -> causal/pad mask
                expS = xh_pool.tile([P, 16, N], sdt, tag="xh",
                                    name=f"expS{s}")
                for kt in range(K):
                    ps = psum.tile([P, N], f32, tag="ps", name=f"pss{s}_{kt}")
                    for g in range(4):
                        nc.tensor.matmul(
                            ps[:], KT[:, 2 * g:2 * g + 2, kt * P:(kt + 1) * P],
                            QT[:, 2 * g:2 * g + 2, s * N:(s + 1) * N],
                            start=(g == 0), stop=(g == 3), perf_mode=DR)
                    nc.scalar.activation(expS[:, kt, :], ps[:], EXP,
                                         scale=SCALE)
                    if (s, kt) in masks:
                        nc.gpsimd.tensor_tensor(
                            out=expS[:, kt, :], in0=expS[:, kt, :],
                            in1=masks[(s, kt)][:], op=MULT)

                # denominator, replicated on all partitions
                dps = psum.tile([P, N], f32, tag="ps", name=f"dps{s}")
                if s == 0:
                    for kt in range(K):
                        nc.tensor.matmul(dps[:], ones[:], expS[:, kt, :],
                                         start=(kt == 0), stop=(kt == K - 1))
                else:
                    for i in range(K // 2):
                        nc.tensor.matmul(
                            dps[:], ones8[:], expS[:, 2 * i:2 * i + 2, :],
                            start=(i == 0), stop=(i == K // 2 - 1),
                            perf_mode=DR)
                rden = rd_pool.tile([P, N], f32)
                nc.vector.reciprocal(rden[:], dps[:])

                # O^T[e,q] with per-et V slabs streamed from DRAM
                for et in range(8):
                    po = psum.tile([P, N], f32, tag="ps", name=f"po{s}_{et}")
                    if s == 0:
                        slab = vin_pool.tile([P, 8, P], bf16, tag="vs",
                                             name=f"vs{s}_{et}")
                        nc.sync.dma_start(slab[:], v_dram[:, et, :K, :])
                        for kt in range(K):
                            nc.tensor.matmul(po[:], slab[:, kt, :],
                                             expS[:, kt, :],
                                             start=(kt == 0),
                                             stop=(kt == K - 1))
                    else:
                        slab8 = v8_pool.tile([P, 16, P], fp8, tag="v8",
                                             name=f"v8_{et}")
                        nc.sync.dma_start(slab8[:], v8_dram[:, et, :, :])
                        for i in range(K // 2):
                            nc.tensor.matmul(
                                po[:], slab8[:, 2 * i:2 * i + 2, :],
                                expS[:, 2 * i:2 * i + 2, :],
                                start=(i == 0), stop=(i == K // 2 - 1),
                                perf_mode=DR)
                    ot = out_pool.tile([P, N], f32)
                    nc.vector.tensor_tensor(out=ot[:], in0=po[:],
                                            in1=rden[:], op=MULT)
                    nc.gpsimd.dma_start(
                        out[et * P:(et + 1) * P, s * N:(s + 1) * N], ot[:])

    _split_multi_waits(nc)
    return nc


_NC_CACHE = None


def _get_nc():
    global _NC_CACHE
    if _NC_CACHE is None:
        _NC_CACHE = build()
    return _NC_CACHE


def _sbufize(a):
    """[rows(1024), cols] -> [dp(128), do(8), cols] contiguous."""
    r, c = a.shape
    return np.ascontiguousarray(a.reshape(8, P, c).transpose(1, 0, 2))


def _host_prep(inputs_for_keys, inputs_for_values, inputs_for_queries,
               weight_q, weight_k, weight_v):
    import ml_dtypes
    bf = ml_dtypes.bfloat16
    f8 = ml_dtypes.float8_e4m3
    f = lambda a, t: np.asarray(a, dtype=np.float32).astype(t)
    ik, iq = f(inputs_for_keys, f8), f(inputs_for_queries, f8)
    iv = f(inputs_for_values, bf)
    iv8 = f(inputs_for_values, f8)
    wq_t = f(weight_q, f8).reshape(8, P, 8, P)      # [d_o, d_p, e_o, e_p]
    wq = np.ascontiguousarray(wq_t.transpose(1, 2, 0, 3))  # [d_p, et, d_o, e]
    wk = _sbufize(f(weight_k, f8))
    wv = _sbufize(f(weight_v, bf))
    wv8 = _sbufize(f(weight_v, f8))

    iota = np.broadcast_to(np.arange(N, dtype=np.float32), (P, N)).copy()
    in_maps = []
    for c in range(NCORES):
        b, h = c // 2, c % 2
        q0s = Q0S[h]
        xq = np.concatenate([iq[b, q0:q0 + 512] for q0 in q0s], axis=0)
        x = np.arange(P, dtype=np.float32)
        thr = np.empty((P, 2, 16), np.float32)
        for s_, q0 in enumerate(q0s):
            for kt in range(16):
                thr[:, s_, kt] = kt * P + x - q0
        in_maps.append({
            "wq": wq, "wk": wk, "wv": wv, "wv8": wv8,
            "xqt": _sbufize(np.ascontiguousarray(xq.T)),
            "xkt": _sbufize(np.ascontiguousarray(ik[b].T)),
            "xvt": _sbufize(np.ascontiguousarray(iv[b, 0:1024].T)),
            "xvt8": _sbufize(np.ascontiguousarray(iv8[b, 1024:2048].T)),
            "thr": thr, "iota": iota,
        })
    return in_maps


def _assemble(results):
    out = np.empty((B, S, D), np.float32)
    for c in range(NCORES):
        b, h = c // 2, c % 2
        oc = results[c]["out"].T        # [q_local, e]
        for s_, q0 in enumerate(Q0S[h]):
            out[b, q0:q0 + 512] = oc[s_ * 512:(s_ + 1) * 512]
    return out


def kernel(**inputs) -> np.ndarray:
    nc = _get_nc()
    in_maps = _host_prep(**inputs)
    res = run_bass_kernel_spmd(nc, in_maps, list(range(NCORES)))
    return _assemble(res.results)


def kernel_profiled(**inputs):
    """Like kernel() but also returns (output, exec_time_ns, results)."""
    nc = _get_nc()
    in_maps = _host_prep(**inputs)
    res = run_bass_kernel_spmd(nc, in_maps, list(range(NCORES)), trace=True)
    return _assemble(res.results), res.exec_time_ns, res



# revision 43
# speedup vs baseline: 1.0259x; 1.0259x over previous
"""Trainium2 Bass kernel for nn_AttentionHead (B=4, S=2048, D_IN=D_OUT=1024).

Sharding: 8 cores; core c handles batch b=c//2 and half the queries,
balanced for causal load: even cores q in [0,512)+[1536,2048), odd cores
q in [512,1536).  Each core computes the full K^T / V projections for its
batch (duplicated within the core pair) plus causal attention for its own
queries, organized as two uniform 512-query phase slots with K_slot=(8,16)
key-tiles.  Causal masking AND the slot padding are data-driven via
host-sent per-partition thresholds (mask = iota >= thr applied to exp(S)),
so all 8 cores run one identical SPMD program.

All matmuls use bf16 operands with fp32 PSUM accumulation (full-rate
TensorE at free-dim 512, and LDWEIGHTS rides fast-weight-load so it
hides under the matmuls; end-to-end rel err ~1e-3).  Everything is
computed transposed so no on-chip transposes
are ever needed:
  stage A: K^T[e,k] = Wk-tiles.T @ Xk^T      (host pre-transposes X into
           SBUF-ready [d_p, d_o, s] blocks; d-outer loop so the first
           matmul needs only one 256KB strip)
  stage C: Q^T[e,q] = Wq-tiles.T @ Xq^T      (Wq rows overwrite the wk
           tile in place; range-based WAR keeps it pipelined)
  stage B: V[k,e]  = Xv^T-tiles.T @ Wv       (staged to DRAM in an
           [k_p, et, k_o, e] layout so stage-D slab reads are contiguous)
  stage D per slot: S^T[k,q] = KT-tiles.T @ Q^T, exp+mask on S^T,
           den = ones.T @ expS (denominator replicated on all partitions),
           O^T[e,q] = V-slab-tiles.T @ expS^T, scaled by 1/den.
Output is O^T per core; the host reassembles [B,S,D].

Perf notes: ~296us HW time on 8 cores (TensorE ~88% busy, matmul p50
230ns at N=512).  DMA queues: bulk loads ride HWDGE (nc.sync), stores +
small loads ride SWDGE (nc.gpsimd) so PE load-waits never sit behind
result-dependent stores; walrus accepts only ONE sync-wait per
instruction, so _split_multi_waits() splits extras onto wait-only NoOps.
"""
import sys
import types

sys.path.insert(0, "/opt/trn_rl_repo")


def _install_ntff_hook():
    try:
        import antenv
    except ImportError:
        return

    if "antenv.axon_hooks" in sys.modules:
        return
    mod = types.ModuleType("antenv.axon_hooks")
    _h = [None]
    mod.set_axon_ntff_profile_hook = lambda h: _h.__setitem__(0, h)
    mod.get_axon_ntff_profile_hook = lambda: _h[0]
    sys.modules["antenv.axon_hooks"] = mod
    antenv.axon_hooks = mod
    try:
        from trn_agent_boot.trn_boot import _ntff_profile_via_ctypes

        mod.set_axon_ntff_profile_hook(
            _ntff_profile_via_ctypes("/opt/axon/libaxon_pjrt.so"))
    except Exception:
        pass


_install_ntff_hook()


import numpy as np
import concourse.bass as bass
import concourse.tile as tile
from concourse import mybir
from concourse.bass_utils import run_bass_kernel_spmd

P = 128
B, S, D = 4, 2048, 1024
N = 512                      # matmul moving free dim / queries per slot
NCORES = 8
K_SLOTS = (8, 16)            # k-tiles per phase slot (uniform across cores)
Q0S = {0: (0, 1536), 1: (512, 1024)}   # slot query starts per core parity
SCALE = float(1.0 / np.sqrt(np.float32(2048)))

f32 = mybir.dt.float32
bf16 = mybir.dt.bfloat16
fp8 = mybir.dt.float8e4
DR = mybir.MatmulPerfMode.DoubleRow
EXP = mybir.ActivationFunctionType.Exp
MULT = mybir.AluOpType.mult


def _split_multi_waits(nc):
    """Walrus allows one sync-wait per instruction; split extras onto
    wait-only NoOps inserted right before the offending instruction."""
    for f in nc.m.functions:
        for bb in f.blocks:
            insts = bb.instructions
            i = 0
            while i < len(insts):
                ins = insts[i]
                si = getattr(ins, "sync_info", None)
                if si and si.on_wait and len(si.on_wait) > 1:
                    waits = list(si.on_wait)
                    for j, w in enumerate(waits[:-1]):
                        nop = mybir.InstNoOp(
                            name=f"{ins.name}-waitsplit-{j}",
                            sync_info=mybir.SyncInfo(on_wait=[w], on_update=[]),
                            bass_nofuse=True,
                            engine=ins.engine, ins=[], outs=[])
                        insts.insert(i + j, nop)
                    i += len(waits) - 1
                    ins.sync_info = mybir.SyncInfo(
                        on_wait=[waits[-1]], on_update=list(si.on_update))
                i += 1


def build():
    nc = bass.Bass()
    # all host-side tensors are pre-arranged into SBUF layout [dp, do, cols]
    wq = nc.dram_tensor("wq", [P, 8, 8, P], fp8, kind="ExternalInput")
    wk = nc.dram_tensor("wk", [P, 8, D], fp8, kind="ExternalInput")
    wv = nc.dram_tensor("wv", [P, 8, D], bf16, kind="ExternalInput")
    wv8 = nc.dram_tensor("wv8", [P, 8, D], fp8, kind="ExternalInput")
    xqt = nc.dram_tensor("xqt", [P, 8, 1024], fp8, kind="ExternalInput")
    xkt = nc.dram_tensor("xkt", [P, 8, S], fp8, kind="ExternalInput")
    xvt = nc.dram_tensor("xvt", [P, 8, 1024], bf16, kind="ExternalInput")
    xvt8 = nc.dram_tensor("xvt8", [P, 8, 1024], fp8, kind="ExternalInput")
    thr = nc.dram_tensor("thr", [P, 2, 16], f32, kind="ExternalInput")
    iot = nc.dram_tensor("iota", [P, N], f32, kind="ExternalInput")
    out = nc.dram_tensor("out", [D, 1024], f32, kind="ExternalOutput")

    with tile.TileContext(nc) as tc:
        from contextlib import ExitStack
        with ExitStack() as ctx:
            kt_pool = ctx.enter_context(tc.tile_pool(name="ktp", bufs=1))
            xh_pool = ctx.enter_context(tc.tile_pool(name="xh", bufs=1))
            qt_pool = ctx.enter_context(tc.tile_pool(name="qtp", bufs=1))
            sm_pool = ctx.enter_context(tc.tile_pool(name="sm", bufs=1))
            psum = ctx.enter_context(
                tc.tile_pool(name="ps", bufs=8, space="PSUM"))
            dram = ctx.enter_context(
                tc.tile_pool(name="dram", bufs=1, space="DRAM"))

            v_dram = dram.tile([P, 8, 16, P], bf16)  # V: [k_p, et, k_o, e]
            v8_dram = dram.tile([P, 8, 16, P], fp8)  # fp8 copy for slot 1
            KT = kt_pool.tile([P, 8, S], fp8)        # K^T: [e_p, e_o, k]
            QT = qt_pool.tile([P, 8, 1024], fp8)     # Q^T: [e_p, e_o, q_loc]

            ones = sm_pool.tile([P, P], bf16)
            nc.gpsimd.memset(ones[:], 1.0)
            ones8 = sm_pool.tile([P, 2, P], fp8)
            nc.gpsimd.memset(ones8[:], 1.0)

            ET_GROUPS = ((0, 2), (2, 4), (4, 6), (6, 8))

            # warm up the PE HAM clock while the first input strips stream in
            wps = psum.tile([P, N], f32, tag="ps", name="warmps")
            for i in range(24):
                nc.tensor.matmul(wps[:, 0:P], ones[:], ones[:],
                                 start=(i == 0), stop=(i == 23))

            def copy_alt(i, dst, src):
                if i % 2 == 0:
                    nc.vector.tensor_copy(dst, src)
                else:
                    nc.scalar.copy(dst, src)

            # w_sb is overwritten in place three times (wk -> wq -> wv);
            # range-based tracking gives per-row WAR deps, so each overwrite
            # streams in while later rows are still being consumed.
            with tc.tile_pool(name="wres", bufs=1) as wres:
                w_sb = wres.tile([P, 8, D], fp8, tag="w")

                # ---- Stage A: K^T[e,k] = sum_d Wk-tiles.T @ Xk^T[d,k] ----
                # loads ride 3 queues in d-pair strips matching DR t-pairs,
                # so descriptor generation (~0.7us/desc/queue) never starves
                # the PE.
                QS = (nc.sync, nc.scalar, nc.gpsimd)
                # double-buffered xk halves: every load below is dep-free, so
                # the queues stream back-to-back with no head-of-line WAR
                # blocking.
                xk_hs = [xh_pool.tile([P, 8, 1024], fp8, tag=f"xk{h}",
                                      name=f"xk{h}") for h in range(2)]
                for d in range(8):
                    QS[d % 3].dma_start(w_sb[:, d, :], wk[:, d, :])
                    QS[(d + 1) % 3].dma_start(xk_hs[0][:, d, :],
                                              xkt[:, d, 0:1024])
                for d in range(8):
                    QS[(d + 2) % 3].dma_start(xk_hs[1][:, d, :],
                                              xkt[:, d, 1024:2048])
                for half in range(2):
                    xk_h = xk_hs[half]
                    for g0, g1 in ET_GROUPS:
                        pss = {}
                        for et in range(g0, g1):
                            for kc in range(2):
                                pss[(et, kc)] = psum.tile(
                                    [P, N], f32, tag="ps",
                                    name=f"psa{half}_{et}_{kc}")
                        for t in range(4):
                            for et in range(g0, g1):
                                lhs = w_sb[:, 2 * t:2 * t + 2,
                                           et * P:(et + 1) * P]
                                for kc in range(2):
                                    nc.tensor.matmul(
                                        pss[(et, kc)][:], lhs,
                                        xk_h[:, 2 * t:2 * t + 2,
                                             kc * N:(kc + 1) * N],
                                        start=(t == 0), stop=(t == 3),
                                        perf_mode=DR)
                        for i, et in enumerate(range(g0, g1)):
                            for kc in range(2):
                                col = half * 1024 + kc * N
                                copy_alt(i + kc, KT[:, et, col:col + N],
                                         pss[(et, kc)][:])

                # ---- Stage C: Q^T[e,q] = sum_d Wq-tiles.T @ Xq^T[d,q] ----
                # wq gets its own tile (no WAR on w_sb) so its loads stream
                # during stage A; wq_sb[:, et, d*P:(d+1)*P] holds
                # Wq[d*P:(d+1)*P, et*P:(et+1)*P]
                wq_sb = wres.tile([P, 8, D], fp8, tag="wq", name="wq_sb")
                for et in range(8):
                    QS[et % 3].dma_start(wq_sb[:, et, :], wq[:, et, :, :])
                with tc.tile_pool(name="xqs", bufs=8) as xq_pool:
                    xqhs = {}
                    for qc in range(2):
                        for t in range(4):
                            xqh = xq_pool.tile([P, 2, N], fp8, tag="xq",
                                               name=f"xq{qc}_{t}")
                            QS[(qc * 4 + t) % 3].dma_start(
                                xqh[:],
                                xqt[:, 2 * t:2 * t + 2, qc * N:(qc + 1) * N])
                            xqhs[(qc, t)] = xqh
                    # 4 PSUM banks per et-group so consecutive groups pipeline
                    for qc in range(2):
                        for eg in range(2):
                            ets = range(4 * eg, 4 * eg + 4)
                            psq = {et: psum.tile([P, N], f32, tag="ps",
                                                 name=f"psq{qc}_{et}")
                                   for et in ets}
                            for t in range(4):
                                for et in ets:
                                    lhs = wq_sb[
                                        :, et, 2 * t * P:(2 * t + 2) * P
                                    ].rearrange("p (two f) -> p two f", two=2)
                                    nc.tensor.matmul(
                                        psq[et][:], lhs, xqhs[(qc, t)][:],
                                        start=(t == 0), stop=(t == 3),
                                        perf_mode=DR)
                            for et in ets:
                                copy_alt(et, QT[:, et, qc * N:(qc + 1) * N],
                                         psq[et][:])

            # ---- Stage B: V[k,e] = sum_d Xv^T-tiles.T @ Wv[d,e] ----
            iota_sb = sm_pool.tile([P, N], f32)
            nc.sync.dma_start(iota_sb[:], iot[:])
            thr_sb = sm_pool.tile([P, 2, 16], f32)
            nc.sync.dma_start(thr_sb[:], thr[:])
            if True:  # keep indent level of the former wres scope
                with tc.tile_pool(name="vp", bufs=3) as v_pool, \
                        tc.tile_pool(name="wvp", bufs=1) as wv_pool:
                    w_sb = wv_pool.tile([P, 8, D], bf16, tag="wv")
                    wv8_sb = wv_pool.tile([P, 8, D], fp8, tag="wv8",
                                          name="wv8_sb")
                    for d in range(8):
                        QS[d % 3].dma_start(w_sb[:, d, :], wv[:, d, :])
                    # half 0 (keys 0..1023, feeds the earliest queries) stays
                    # bf16; half 1 (keys 1024..2047, only ever attended with
                    # n_eff >= 1024) runs fp8 DoubleRow.
                    xv_h0 = xh_pool.tile([P, 8, 1024], bf16, tag="xv0",
                                         name="xv0")
                    xv_h1 = xh_pool.tile([P, 8, 1024], fp8, tag="xv1",
                                         name="xv1")
                    for d in range(8):
                        QS[(d + 1) % 3].dma_start(xv_h0[:, d, :],
                                                  xvt[:, d, :])
                    for d in range(8):
                        QS[(d + 2) % 3].dma_start(xv_h1[:, d, :],
                                                  xvt8[:, d, :])
                        QS[d % 3].dma_start(wv8_sb[:, d, :], wv8[:, d, :])
                    for half in range(2):
                        for g0, g1 in ((0, 2), (2, 4), (4, 6), (6, 8)):
                            ps2 = {}
                            for ktl in range(g0, g1):
                                for ec in range(2):
                                    ps2[(ktl, ec)] = psum.tile(
                                        [P, N], f32, tag="ps",
                                        name=f"psb{half}_{ktl}_{ec}")
                            if half == 0:
                                for d in range(8):
                                    for ktl in range(g0, g1):
                                        lhs = xv_h0[:, d,
                                                    ktl * P:(ktl + 1) * P]
                                        for ec in range(2):
                                            nc.tensor.matmul(
                                                ps2[(ktl, ec)][:], lhs,
                                                w_sb[:, d,
                                                     ec * N:(ec + 1) * N],
                                                start=(d == 0), stop=(d == 7))
                            else:
                                for t in range(4):
                                    for ktl in range(g0, g1):
                                        lhs = xv_h1[:, 2 * t:2 * t + 2,
                                                    ktl * P:(ktl + 1) * P]
                                        for ec in range(2):
                                            nc.tensor.matmul(
                                                ps2[(ktl, ec)][:], lhs,
                                                wv8_sb[:, 2 * t:2 * t + 2,
                                                       ec * N:(ec + 1) * N],
                                                start=(t == 0), stop=(t == 3),
                                                perf_mode=DR)
                            for ktl in range(g0, g1):
                                ktg = half * 8 + ktl
                                for ec in range(2):
                                    vt8 = v_pool.tile([P, N], fp8,
                                                      tag="vst8", name="vt8")
                                    if half == 0:
                                        # slot 0 needs bf16 V; slot 1 reads
                                        # the fp8 copy (cast on idle DVE)
                                        vt = v_pool.tile([P, N], bf16,
                                                         tag="vst")
                                        nc.scalar.copy(vt[:],
                                                       ps2[(ktl, ec)][:])
                                        nc.vector.tensor_copy(vt8[:], vt[:])
                                        nc.scalar.dma_start(
                                            v_dram[:, ec * 4:(ec + 1) * 4,
                                                   ktg, :],
                                            vt[:].rearrange(
                                                "p (et e) -> p et e", e=P))
                                    else:
                                        nc.scalar.copy(vt8[:],
                                                       ps2[(ktl, ec)][:])
                                    nc.gpsimd.dma_start(
                                        v8_dram[:, ec * 4:(ec + 1) * 4,
                                                ktg, :],
                                        vt8[:].rearrange(
                                            "p (et e) -> p et e", e=P))

            # ---- Stage D: per phase slot: scores, softmax, O^T ----
            # masks precomputed on DVE (overlaps stage B); the per-kt apply
            # rides the otherwise-idle gpsimd engine.
            vin_pool = ctx.enter_context(tc.tile_pool(name="vin", bufs=6))
            out_pool = ctx.enter_context(tc.tile_pool(name="op", bufs=3))
            mk_pool = ctx.enter_context(tc.tile_pool(name="mk", bufs=1))
            rd_pool = ctx.enter_context(tc.tile_pool(name="rd", bufs=2))
            v8_pool = ctx.enter_context(tc.tile_pool(name="v8", bufs=3))
            masks = {}
            for s in range(2):
                mdt = bf16 if s == 0 else fp8
                for kt in range(K_SLOTS[s]):
                    if s == 1 and kt < 8:
                        continue
                    mk = mk_pool.tile([P, N], mdt, tag=f"mk{s}_{kt}",
                                      name=f"mk{s}_{kt}")
                    nc.vector.tensor_scalar(
                        out=mk[:], in0=iota_sb[:],
                        scalar1=thr_sb[:, s, kt:kt + 1], scalar2=None,
                        op0=mybir.AluOpType.is_ge)
                    masks[(s, kt)] = mk
            for s in range(2):
                K = K_SLOTS[s]
                # slot 0 holds the earliest queries -> bf16 attention weights
                # and V; slot 1 (n_eff >= 1024 keys) runs fp8 end-to-end.
                sdt = bf16 if s == 0 else fp8
                # scores^T -> exp -> causal/pad mask
                expS = xh_pool.tile([P, 16, N], sdt, tag="xh",
                                    name=f"expS{s}")
                for kt in range(K):
                    ps = psum.tile([P, N], f32, tag="ps", name=f"pss{s}_{kt}")
                    for g in range(4):
                        nc.tensor.matmul(
                            ps[:], KT[:, 2 * g:2 * g + 2, kt * P:(kt + 1) * P],
                            QT[:, 2 * g:2 * g + 2, s * N:(s + 1) * N],
                            start=(g == 0), stop=(g == 3), perf_mode=DR)
                    nc.scalar.activation(expS[:, kt, :], ps[:], EXP,
                                         scale=SCALE)
                    if (s, kt) in masks:
                        nc.vector.tensor_tensor(
                            out=expS[:, kt, :], in0=expS[:, kt, :],
                            in1=masks[(s, kt)][:], op=MULT)

                # denominator, replicated on all partitions
                dps = psum.tile([P, N], f32, tag="ps", name=f"dps{s}")
                if s == 0:
                    for kt in range(K):
                        nc.tensor.matmul(dps[:], ones[:], expS[:, kt, :],
                                         start=(kt == 0), stop=(kt == K - 1))
                else:
                    for i in range(K // 2):
                        nc.tensor.matmul(
                            dps[:], ones8[:], expS[:, 2 * i:2 * i + 2, :],
                            start=(i == 0), stop=(i == K // 2 - 1),
                            perf_mode=DR)
                rden = rd_pool.tile([P, N], f32)
                nc.vector.reciprocal(rden[:], dps[:])

                # O^T[e,q] with per-et V slabs streamed from DRAM
                for et in range(8):
                    po = psum.tile([P, N], f32, tag="ps", name=f"po{s}_{et}")
                    if s == 0:
                        slab = vin_pool.tile([P, 8, P], bf16, tag="vs",
                                             name=f"vs{s}_{et}")
                        nc.sync.dma_start(slab[:], v_dram[:, et, :K, :])
                        for kt in range(K):
                            nc.tensor.matmul(po[:], slab[:, kt, :],
                                             expS[:, kt, :],
                                             start=(kt == 0),
                                             stop=(kt == K - 1))
                    else:
                        slab8 = v8_pool.tile([P, 16, P], fp8, tag="v8",
                                             name=f"v8_{et}")
                        nc.sync.dma_start(slab8[:], v8_dram[:, et, :, :])
                        for i in range(K // 2):
                            nc.tensor.matmul(
                                po[:], slab8[:, 2 * i:2 * i + 2, :],
                                expS[:, 2 * i:2 * i + 2, :],
                                start=(i == 0), stop=(i == K // 2 - 1),
                                perf_mode=DR)
                    ot = out_pool.tile([P, N], f32)
                    nc.vector.tensor_tensor(out=ot[:], in0=po[:],
                                            in1=rden[:], op=MULT)
                    nc.gpsimd.dma_start(
                        out[et * P:(et + 1) * P, s * N:(s + 1) * N], ot[:])

    _split_multi_waits(nc)
    return nc


_NC_CACHE = None


def _get_nc():
    global _NC_CACHE
    if _NC_CACHE is None:
        _NC_CACHE = build()
    return _NC_CACHE


def _sbufize(a):
    """[rows(1024), cols] -> [dp(128), do(8), cols] contiguous."""
    r, c = a.shape
    return np.ascontiguousarray(a.reshape(8, P, c).transpose(1, 0, 2))


def _host_prep(inputs_for_keys, inputs_for_values, inputs_for_queries,
               weight_q, weight_k, weight_v):
    import ml_dtypes
    bf = ml_dtypes.bfloat16
    f8 = ml_dtypes.float8_e4m3
    f = lambda a, t: np.asarray(a, dtype=np.float32).astype(t)
    ik, iq = f(inputs_for_keys, f8), f(inputs_for_queries, f8)
    iv = f(inputs_for_values, bf)
    iv8 = f(inputs_for_values, f8)
    wq_t = f(weight_q, f8).reshape(8, P, 8, P)      # [d_o, d_p, e_o, e_p]
    wq = np.ascontiguousarray(wq_t.transpose(1, 2, 0, 3))  # [d_p, et, d_o, e]
    wk = _sbufize(f(weight_k, f8))
    wv = _sbufize(f(weight_v, bf))
    wv8 = _sbufize(f(weight_v, f8))

    iota = np.broadcast_to(np.arange(N, dtype=np.float32), (P, N)).copy()
    in_maps = []
    for c in range(NCORES):
        b, h = c // 2, c % 2
        q0s = Q0S[h]
        xq = np.concatenate([iq[b, q0:q0 + 512] for q0 in q0s], axis=0)
        x = np.arange(P, dtype=np.float32)
        thr = np.empty((P, 2, 16), np.float32)
        for s_, q0 in enumerate(q0s):
            for kt in range(16):
                thr[:, s_, kt] = kt * P + x - q0
        in_maps.append({
            "wq": wq, "wk": wk, "wv": wv, "wv8": wv8,
            "xqt": _sbufize(np.ascontiguousarray(xq.T)),
            "xkt": _sbufize(np.ascontiguousarray(ik[b].T)),
            "xvt": _sbufize(np.ascontiguousarray(iv[b, 0:1024].T)),
            "xvt8": _sbufize(np.ascontiguousarray(iv8[b, 1024:2048].T)),
            "thr": thr, "iota": iota,
        })
    return in_maps


def _assemble(results):
    out = np.empty((B, S, D), np.float32)
    for c in range(NCORES):
        b, h = c // 2, c % 2
        oc = results[c]["out"].T        # [q_local, e]
        for s_, q0 in enumerate(Q0S[h]):
            out[b, q0:q0 + 512] = oc[s_ * 512:(s_ + 1) * 512]
    return out


def kernel(**inputs) -> np.ndarray:
    nc = _get_nc()
    in_maps = _host_prep(**inputs)
    res = run_bass_kernel_spmd(nc, in_maps, list(range(NCORES)))
    return _assemble(res.results)


def kernel_profiled(**inputs):
    """Like kernel() but also returns (output, exec_time_ns, results)."""
    nc = _get_nc()
    in_maps = _host_prep(**inputs)
    res = run_bass_kernel_spmd(nc, in_maps, list(range(NCORES)), trace=True)
    return _assemble(res.results), res.exec_time_ns, res



# revision 47
# speedup vs baseline: 1.0325x; 1.0064x over previous
"""Trainium2 Bass kernel for nn_AttentionHead (B=4, S=2048, D_IN=D_OUT=1024).

Sharding: 8 cores; core c handles batch b=c//2 and half the queries,
balanced for causal load: even cores q in [0,512)+[1536,2048), odd cores
q in [512,1536).  Each core computes the full K^T / V projections for its
batch (duplicated within the core pair) plus causal attention for its own
queries, organized as two uniform 512-query phase slots with K_slot=(8,16)
key-tiles.  Causal masking AND the slot padding are data-driven via
host-sent per-partition thresholds (mask = iota >= thr applied to exp(S)),
so all 8 cores run one identical SPMD program.

All matmuls use bf16 operands with fp32 PSUM accumulation (full-rate
TensorE at free-dim 512, and LDWEIGHTS rides fast-weight-load so it
hides under the matmuls; end-to-end rel err ~1e-3).  Everything is
computed transposed so no on-chip transposes
are ever needed:
  stage A: K^T[e,k] = Wk-tiles.T @ Xk^T      (host pre-transposes X into
           SBUF-ready [d_p, d_o, s] blocks; d-outer loop so the first
           matmul needs only one 256KB strip)
  stage C: Q^T[e,q] = Wq-tiles.T @ Xq^T      (Wq rows overwrite the wk
           tile in place; range-based WAR keeps it pipelined)
  stage B: V[k,e]  = Xv^T-tiles.T @ Wv       (staged to DRAM in an
           [k_p, et, k_o, e] layout so stage-D slab reads are contiguous)
  stage D per slot: S^T[k,q] = KT-tiles.T @ Q^T, exp+mask on S^T,
           den = ones.T @ expS (denominator replicated on all partitions),
           O^T[e,q] = V-slab-tiles.T @ expS^T, scaled by 1/den.
Output is O^T per core; the host reassembles [B,S,D].

Perf notes: ~296us HW time on 8 cores (TensorE ~88% busy, matmul p50
230ns at N=512).  DMA queues: bulk loads ride HWDGE (nc.sync), stores +
small loads ride SWDGE (nc.gpsimd) so PE load-waits never sit behind
result-dependent stores; walrus accepts only ONE sync-wait per
instruction, so _split_multi_waits() splits extras onto wait-only NoOps.
"""
import sys
import types

sys.path.insert(0, "/opt/trn_rl_repo")


def _install_ntff_hook():
    try:
        import antenv
    except ImportError:
        return

    if "antenv.axon_hooks" in sys.modules:
        return
    mod = types.ModuleType("antenv.axon_hooks")
    _h = [None]
    mod.set_axon_ntff_profile_hook = lambda h: _h.__setitem__(0, h)
    mod.get_axon_ntff_profile_hook = lambda: _h[0]
    sys.modules["antenv.axon_hooks"] = mod
    antenv.axon_hooks = mod
    try:
        from trn_agent_boot.trn_boot import _ntff_profile_via_ctypes

        mod.set_axon_ntff_profile_hook(
            _ntff_profile_via_ctypes("/opt/axon/libaxon_pjrt.so"))
    except Exception:
        pass


_install_ntff_hook()


import numpy as np
import concourse.bass as bass
import concourse.tile as tile
from concourse import mybir
from concourse.bass_utils import run_bass_kernel_spmd

P = 128
B, S, D = 4, 2048, 1024
N = 512                      # matmul moving free dim / queries per slot
NCORES = 8
K_SLOTS = (8, 16)            # k-tiles per phase slot (uniform across cores)
Q0S = {0: (0, 1536), 1: (512, 1024)}   # slot query starts per core parity
SCALE = float(1.0 / np.sqrt(np.float32(2048)))

f32 = mybir.dt.float32
bf16 = mybir.dt.bfloat16
fp8 = mybir.dt.float8e4
DR = mybir.MatmulPerfMode.DoubleRow
EXP = mybir.ActivationFunctionType.Exp
MULT = mybir.AluOpType.mult


def _split_multi_waits(nc):
    """Walrus allows one sync-wait per instruction; split extras onto
    wait-only NoOps inserted right before the offending instruction."""
    for f in nc.m.functions:
        for bb in f.blocks:
            insts = bb.instructions
            i = 0
            while i < len(insts):
                ins = insts[i]
                si = getattr(ins, "sync_info", None)
                if si and si.on_wait and len(si.on_wait) > 1:
                    waits = list(si.on_wait)
                    for j, w in enumerate(waits[:-1]):
                        nop = mybir.InstNoOp(
                            name=f"{ins.name}-waitsplit-{j}",
                            sync_info=mybir.SyncInfo(on_wait=[w], on_update=[]),
                            bass_nofuse=True,
                            engine=ins.engine, ins=[], outs=[])
                        insts.insert(i + j, nop)
                    i += len(waits) - 1
                    ins.sync_info = mybir.SyncInfo(
                        on_wait=[waits[-1]], on_update=list(si.on_update))
                i += 1


def build():
    nc = bass.Bass()
    # all host-side tensors are pre-arranged into SBUF layout [dp, do, cols]
    wq = nc.dram_tensor("wq", [P, 8, 8, P], fp8, kind="ExternalInput")
    wk = nc.dram_tensor("wk", [P, 8, D], fp8, kind="ExternalInput")
    wv = nc.dram_tensor("wv", [P, 8, D], bf16, kind="ExternalInput")
    wv8 = nc.dram_tensor("wv8", [P, 8, D], fp8, kind="ExternalInput")
    xqt = nc.dram_tensor("xqt", [P, 8, 1024], fp8, kind="ExternalInput")
    xkt = nc.dram_tensor("xkt", [P, 8, S], fp8, kind="ExternalInput")
    xvt = nc.dram_tensor("xvt", [P, 8, 1024], bf16, kind="ExternalInput")
    xvt8 = nc.dram_tensor("xvt8", [P, 8, 1024], fp8, kind="ExternalInput")
    thr = nc.dram_tensor("thr", [P, 2, 16], f32, kind="ExternalInput")
    iot = nc.dram_tensor("iota", [P, N], f32, kind="ExternalInput")
    out = nc.dram_tensor("out", [D, 1024], f32, kind="ExternalOutput")

    with tile.TileContext(nc) as tc:
        from contextlib import ExitStack
        with ExitStack() as ctx:
            kt_pool = ctx.enter_context(tc.tile_pool(name="ktp", bufs=1))
            xh_pool = ctx.enter_context(tc.tile_pool(name="xh", bufs=1))
            qt_pool = ctx.enter_context(tc.tile_pool(name="qtp", bufs=1))
            sm_pool = ctx.enter_context(tc.tile_pool(name="sm", bufs=1))
            psum = ctx.enter_context(
                tc.tile_pool(name="ps", bufs=8, space="PSUM"))
            dram = ctx.enter_context(
                tc.tile_pool(name="dram", bufs=1, space="DRAM"))

            v_dram = dram.tile([P, 8, 16, P], bf16)  # V: [k_p, et, k_o, e]
            v8_dram = dram.tile([P, 8, 16, P], fp8)  # fp8 copy for slot 1
            KT = kt_pool.tile([P, 8, S], fp8)        # K^T: [e_p, e_o, k]
            QT = qt_pool.tile([P, 8, 1024], fp8)     # Q^T: [e_p, e_o, q_loc]

            ones = sm_pool.tile([P, P], bf16)
            nc.gpsimd.memset(ones[:], 1.0)
            ones8 = sm_pool.tile([P, 2, P], fp8)
            nc.gpsimd.memset(ones8[:], 1.0)

            ET_GROUPS = ((0, 2), (2, 4), (4, 6), (6, 8))

            # warm up the PE HAM clock while the first input strips stream in
            wps = psum.tile([P, N], f32, tag="ps", name="warmps")
            for i in range(24):
                nc.tensor.matmul(wps[:, 0:P], ones[:], ones[:],
                                 start=(i == 0), stop=(i == 23))

            def copy_alt(i, dst, src):
                if i % 2 == 0:
                    nc.vector.tensor_copy(dst, src)
                else:
                    nc.scalar.copy(dst, src)

            # w_sb is overwritten in place three times (wk -> wq -> wv);
            # range-based tracking gives per-row WAR deps, so each overwrite
            # streams in while later rows are still being consumed.
            with tc.tile_pool(name="wres", bufs=1) as wres:
                w_sb = wres.tile([P, 8, D], fp8, tag="w")

                # ---- Stage A: K^T[e,k] = sum_d Wk-tiles.T @ Xk^T[d,k] ----
                # loads ride 3 queues in d-pair strips matching DR t-pairs,
                # so descriptor generation (~0.7us/desc/queue) never starves
                # the PE.
                QS = (nc.sync, nc.scalar, nc.gpsimd)
                # double-buffered xk halves: every load below is dep-free, so
                # the queues stream back-to-back with no head-of-line WAR
                # blocking.
                xk_hs = [xh_pool.tile([P, 8, 1024], fp8, tag=f"xk{h}",
                                      name=f"xk{h}") for h in range(2)]
                for d in range(8):
                    QS[d % 3].dma_start(w_sb[:, d, :], wk[:, d, :])
                    QS[(d + 1) % 3].dma_start(xk_hs[0][:, d, :],
                                              xkt[:, d, 0:1024])
                for d in range(8):
                    QS[(d + 2) % 3].dma_start(xk_hs[1][:, d, :],
                                              xkt[:, d, 1024:2048])
                for half in range(2):
                    xk_h = xk_hs[half]
                    for g0, g1 in ET_GROUPS:
                        pss = {}
                        for et in range(g0, g1):
                            for kc in range(2):
                                pss[(et, kc)] = psum.tile(
                                    [P, N], f32, tag="ps",
                                    name=f"psa{half}_{et}_{kc}")
                        for t in range(4):
                            for et in range(g0, g1):
                                lhs = w_sb[:, 2 * t:2 * t + 2,
                                           et * P:(et + 1) * P]
                                for kc in range(2):
                                    nc.tensor.matmul(
                                        pss[(et, kc)][:], lhs,
                                        xk_h[:, 2 * t:2 * t + 2,
                                             kc * N:(kc + 1) * N],
                                        start=(t == 0), stop=(t == 3),
                                        perf_mode=DR)
                        for i, et in enumerate(range(g0, g1)):
                            for kc in range(2):
                                col = half * 1024 + kc * N
                                copy_alt(i + kc, KT[:, et, col:col + N],
                                         pss[(et, kc)][:])

                # ---- Stage C: Q^T[e,q] = sum_d Wq-tiles.T @ Xq^T[d,q] ----
                # wq gets its own tile (no WAR on w_sb) so its loads stream
                # during stage A; wq_sb[:, et, d*P:(d+1)*P] holds
                # Wq[d*P:(d+1)*P, et*P:(et+1)*P]
                wq_sb = wres.tile([P, 8, D], fp8, tag="wq", name="wq_sb")
                for et in range(8):
                    QS[et % 3].dma_start(wq_sb[:, et, :], wq[:, et, :, :])
                with tc.tile_pool(name="xqs", bufs=8) as xq_pool:
                    xqhs = {}
                    for qc in range(2):
                        for t in range(4):
                            xqh = xq_pool.tile([P, 2, N], fp8, tag="xq",
                                               name=f"xq{qc}_{t}")
                            QS[(qc * 4 + t) % 3].dma_start(
                                xqh[:],
                                xqt[:, 2 * t:2 * t + 2, qc * N:(qc + 1) * N])
                            xqhs[(qc, t)] = xqh
                    # 4 PSUM banks per et-group so consecutive groups pipeline
                    for qc in range(2):
                        for eg in range(2):
                            ets = range(4 * eg, 4 * eg + 4)
                            psq = {et: psum.tile([P, N], f32, tag="ps",
                                                 name=f"psq{qc}_{et}")
                                   for et in ets}
                            for t in range(4):
                                for et in ets:
                                    lhs = wq_sb[
                                        :, et, 2 * t * P:(2 * t + 2) * P
                                    ].rearrange("p (two f) -> p two f", two=2)
                                    nc.tensor.matmul(
                                        psq[et][:], lhs, xqhs[(qc, t)][:],
                                        start=(t == 0), stop=(t == 3),
                                        perf_mode=DR)
                            for et in ets:
                                copy_alt(et, QT[:, et, qc * N:(qc + 1) * N],
                                         psq[et][:])

            # ---- Stage B: V[k,e] = sum_d Xv^T-tiles.T @ Wv[d,e] ----
            iota_sb = sm_pool.tile([P, N], f32)
            nc.sync.dma_start(iota_sb[:], iot[:])
            thr_sb = sm_pool.tile([P, 2, 16], f32)
            nc.sync.dma_start(thr_sb[:], thr[:])
            # stage-D masks precomputed here: DVE is idle during stage B
            mk_pool = ctx.enter_context(tc.tile_pool(name="mk", bufs=1))
            masks = {}
            for s in range(2):
                mdt = bf16 if s == 0 else fp8
                for kt in range(K_SLOTS[s]):
                    if s == 1 and kt < 8:
                        continue
                    mk = mk_pool.tile([P, N], mdt, tag=f"mk{s}_{kt}",
                                      name=f"mk{s}_{kt}")
                    nc.vector.tensor_scalar(
                        out=mk[:], in0=iota_sb[:],
                        scalar1=thr_sb[:, s, kt:kt + 1], scalar2=None,
                        op0=mybir.AluOpType.is_ge)
                    masks[(s, kt)] = mk
            if True:  # keep indent level of the former wres scope
                with tc.tile_pool(name="vp", bufs=3) as v_pool, \
                        tc.tile_pool(name="wvp", bufs=1) as wv_pool:
                    w_sb = wv_pool.tile([P, 8, D], bf16, tag="wv")
                    wv8_sb = wv_pool.tile([P, 8, D], fp8, tag="wv8",
                                          name="wv8_sb")
                    for d in range(8):
                        QS[d % 3].dma_start(w_sb[:, d, :], wv[:, d, :])
                    # half 0 (keys 0..1023, feeds the earliest queries) stays
                    # bf16; half 1 (keys 1024..2047, only ever attended with
                    # n_eff >= 1024) runs fp8 DoubleRow.
                    xv_h0 = xh_pool.tile([P, 8, 1024], bf16, tag="xv0",
                                         name="xv0")
                    xv_h1 = xh_pool.tile([P, 8, 1024], fp8, tag="xv1",
                                         name="xv1")
                    for d in range(8):
                        QS[(d + 1) % 3].dma_start(xv_h0[:, d, :],
                                                  xvt[:, d, :])
                    for d in range(8):
                        QS[(d + 2) % 3].dma_start(xv_h1[:, d, :],
                                                  xvt8[:, d, :])
                        QS[d % 3].dma_start(wv8_sb[:, d, :], wv8[:, d, :])
                    for half in range(2):
                        for g0, g1 in ((0, 2), (2, 4), (4, 6), (6, 8)):
                            ps2 = {}
                            for ktl in range(g0, g1):
                                for ec in range(2):
                                    ps2[(ktl, ec)] = psum.tile(
                                        [P, N], f32, tag="ps",
                                        name=f"psb{half}_{ktl}_{ec}")
                            if half == 0:
                                for d in range(8):
                                    for ktl in range(g0, g1):
                                        lhs = xv_h0[:, d,
                                                    ktl * P:(ktl + 1) * P]
                                        for ec in range(2):
                                            nc.tensor.matmul(
                                                ps2[(ktl, ec)][:], lhs,
                                                w_sb[:, d,
                                                     ec * N:(ec + 1) * N],
                                                start=(d == 0), stop=(d == 7))
                            else:
                                for t in range(4):
                                    for ktl in range(g0, g1):
                                        lhs = xv_h1[:, 2 * t:2 * t + 2,
                                                    ktl * P:(ktl + 1) * P]
                                        for ec in range(2):
                                            nc.tensor.matmul(
                                                ps2[(ktl, ec)][:], lhs,
                                                wv8_sb[:, 2 * t:2 * t + 2,
                                                       ec * N:(ec + 1) * N],
                                                start=(t == 0), stop=(t == 3),
                                                perf_mode=DR)
                            for ktl in range(g0, g1):
                                ktg = half * 8 + ktl
                                for ec in range(2):
                                    vt8 = v_pool.tile([P, N], fp8,
                                                      tag="vst8", name="vt8")
                                    if half == 0:
                                        # slot 0 needs bf16 V; slot 1 reads
                                        # the fp8 copy (cast on idle DVE)
                                        vt = v_pool.tile([P, N], bf16,
                                                         tag="vst")
                                        nc.scalar.copy(vt[:],
                                                       ps2[(ktl, ec)][:])
                                        nc.vector.tensor_copy(vt8[:], vt[:])
                                        nc.scalar.dma_start(
                                            v_dram[:, ec * 4:(ec + 1) * 4,
                                                   ktg, :],
                                            vt[:].rearrange(
                                                "p (et e) -> p et e", e=P))
                                    else:
                                        nc.scalar.copy(vt8[:],
                                                       ps2[(ktl, ec)][:])
                                    nc.gpsimd.dma_start(
                                        v8_dram[:, ec * 4:(ec + 1) * 4,
                                                ktg, :],
                                        vt8[:].rearrange(
                                            "p (et e) -> p et e", e=P))

            # ---- Stage D: per phase slot: scores, softmax, O^T ----
            # masks precomputed on DVE (overlaps stage B); the per-kt apply
            # rides the otherwise-idle gpsimd engine.
            vin_pool = ctx.enter_context(tc.tile_pool(name="vin", bufs=6))
            out_pool = ctx.enter_context(tc.tile_pool(name="op", bufs=3))
            rd_pool = ctx.enter_context(tc.tile_pool(name="rd", bufs=2))
            v8_pool = ctx.enter_context(tc.tile_pool(name="v8", bufs=3))
            for s in range(2):
                K = K_SLOTS[s]
                # slot 0 holds the earliest queries -> bf16 attention weights
                # and V; slot 1 (n_eff >= 1024 keys) runs fp8 end-to-end.
                sdt = bf16 if s == 0 else fp8
                # scores^T -> exp -> causal/pad mask
                expS = xh_pool.tile([P, 16, N], sdt, tag="xh",
                                    name=f"expS{s}")
                for kt in range(K):
                    ps = psum.tile([P, N], f32, tag="ps", name=f"pss{s}_{kt}")
                    for g in range(4):
                        nc.tensor.matmul(
                            ps[:], KT[:, 2 * g:2 * g + 2, kt * P:(kt + 1) * P],
                            QT[:, 2 * g:2 * g + 2, s * N:(s + 1) * N],
                            start=(g == 0), stop=(g == 3), perf_mode=DR)
                    nc.scalar.activation(expS[:, kt, :], ps[:], EXP,
                                         scale=SCALE)
                    if (s, kt) in masks:
                        nc.vector.tensor_tensor(
                            out=expS[:, kt, :], in0=expS[:, kt, :],
                            in1=masks[(s, kt)][:], op=MULT)

                # denominator, replicated on all partitions
                dps = psum.tile([P, N], f32, tag="ps", name=f"dps{s}")
                if s == 0:
                    for kt in range(K):
                        nc.tensor.matmul(dps[:], ones[:], expS[:, kt, :],
                                         start=(kt == 0), stop=(kt == K - 1))
                else:
                    for i in range(K // 2):
                        nc.tensor.matmul(
                            dps[:], ones8[:], expS[:, 2 * i:2 * i + 2, :],
                            start=(i == 0), stop=(i == K // 2 - 1),
                            perf_mode=DR)
                rden = rd_pool.tile([P, N], f32)
                nc.vector.reciprocal(rden[:], dps[:])

                # O^T[e,q] with per-et V slabs streamed from DRAM
                for et in range(8):
                    po = psum.tile([P, N], f32, tag="ps", name=f"po{s}_{et}")
                    if s == 0:
                        slab = vin_pool.tile([P, 8, P], bf16, tag="vs",
                                             name=f"vs{s}_{et}")
                        nc.sync.dma_start(slab[:], v_dram[:, et, :K, :])
                        for kt in range(K):
                            nc.tensor.matmul(po[:], slab[:, kt, :],
                                             expS[:, kt, :],
                                             start=(kt == 0),
                                             stop=(kt == K - 1))
                    else:
                        slab8 = v8_pool.tile([P, 16, P], fp8, tag="v8",
                                             name=f"v8_{et}")
                        nc.sync.dma_start(slab8[:], v8_dram[:, et, :, :])
                        for i in range(K // 2):
                            nc.tensor.matmul(
                                po[:], slab8[:, 2 * i:2 * i + 2, :],
                                expS[:, 2 * i:2 * i + 2, :],
                                start=(i == 0), stop=(i == K // 2 - 1),
                                perf_mode=DR)
                    ot = out_pool.tile([P, N], f32)
                    nc.vector.tensor_tensor(out=ot[:], in0=po[:],
                                            in1=rden[:], op=MULT)
                    nc.gpsimd.dma_start(
                        out[et * P:(et + 1) * P, s * N:(s + 1) * N], ot[:])

    _split_multi_waits(nc)
    return nc


_NC_CACHE = None


def _get_nc():
    global _NC_CACHE
    if _NC_CACHE is None:
        _NC_CACHE = build()
    return _NC_CACHE


def _sbufize(a):
    """[rows(1024), cols] -> [dp(128), do(8), cols] contiguous."""
    r, c = a.shape
    return np.ascontiguousarray(a.reshape(8, P, c).transpose(1, 0, 2))


def _host_prep(inputs_for_keys, inputs_for_values, inputs_for_queries,
               weight_q, weight_k, weight_v):
    import ml_dtypes
    bf = ml_dtypes.bfloat16
    f8 = ml_dtypes.float8_e4m3
    f = lambda a, t: np.asarray(a, dtype=np.float32).astype(t)
    ik, iq = f(inputs_for_keys, f8), f(inputs_for_queries, f8)
    iv = f(inputs_for_values, bf)
    iv8 = f(inputs_for_values, f8)
    wq_t = f(weight_q, f8).reshape(8, P, 8, P)      # [d_o, d_p, e_o, e_p]
    wq = np.ascontiguousarray(wq_t.transpose(1, 2, 0, 3))  # [d_p, et, d_o, e]
    wk = _sbufize(f(weight_k, f8))
    wv = _sbufize(f(weight_v, bf))
    wv8 = _sbufize(f(weight_v, f8))

    iota = np.broadcast_to(np.arange(N, dtype=np.float32), (P, N)).copy()
    in_maps = []
    for c in range(NCORES):
        b, h = c // 2, c % 2
        q0s = Q0S[h]
        xq = np.concatenate([iq[b, q0:q0 + 512] for q0 in q0s], axis=0)
        x = np.arange(P, dtype=np.float32)
        thr = np.empty((P, 2, 16), np.float32)
        for s_, q0 in enumerate(q0s):
            for kt in range(16):
                thr[:, s_, kt] = kt * P + x - q0
        in_maps.append({
            "wq": wq, "wk": wk, "wv": wv, "wv8": wv8,
            "xqt": _sbufize(np.ascontiguousarray(xq.T)),
            "xkt": _sbufize(np.ascontiguousarray(ik[b].T)),
            "xvt": _sbufize(np.ascontiguousarray(iv[b, 0:1024].T)),
            "xvt8": _sbufize(np.ascontiguousarray(iv8[b, 1024:2048].T)),
            "thr": thr, "iota": iota,
        })
    return in_maps


def _assemble(results):
    out = np.empty((B, S, D), np.float32)
    for c in range(NCORES):
        b, h = c // 2, c % 2
        oc = results[c]["out"].T        # [q_local, e]
        for s_, q0 in enumerate(Q0S[h]):
            out[b, q0:q0 + 512] = oc[s_ * 512:(s_ + 1) * 512]
    return out


def kernel(**inputs) -> np.ndarray:
    nc = _get_nc()
    in_maps = _host_prep(**inputs)
    res = run_bass_kernel_spmd(nc, in_maps, list(range(NCORES)))
    return _assemble(res.results)


def kernel_profiled(**inputs):
    """Like kernel() but also returns (output, exec_time_ns, results)."""
    nc = _get_nc()
    in_maps = _host_prep(**inputs)
    res = run_bass_kernel_spmd(nc, in_maps, list(range(NCORES)), trace=True)
    return _assemble(res.results), res.exec_time_ns, res



# revision 51
# speedup vs baseline: 1.0393x; 1.0067x over previous
"""Trainium2 Bass kernel for nn_AttentionHead (B=4, S=2048, D_IN=D_OUT=1024).

Sharding: 8 cores; core c handles batch b=c//2 and half the queries,
balanced for causal load: even cores q in [0,512)+[1536,2048), odd cores
q in [512,1536).  Each core computes the full K^T / V projections for its
batch (duplicated within the core pair) plus causal attention for its own
queries, organized as two uniform 512-query phase slots with K_slot=(8,16)
key-tiles.  Causal masking AND the slot padding are data-driven via
host-sent per-partition thresholds (mask = iota >= thr applied to exp(S)),
so all 8 cores run one identical SPMD program.

All matmuls use bf16 operands with fp32 PSUM accumulation (full-rate
TensorE at free-dim 512, and LDWEIGHTS rides fast-weight-load so it
hides under the matmuls; end-to-end rel err ~1e-3).  Everything is
computed transposed so no on-chip transposes
are ever needed:
  stage A: K^T[e,k] = Wk-tiles.T @ Xk^T      (host pre-transposes X into
           SBUF-ready [d_p, d_o, s] blocks; d-outer loop so the first
           matmul needs only one 256KB strip)
  stage C: Q^T[e,q] = Wq-tiles.T @ Xq^T      (Wq rows overwrite the wk
           tile in place; range-based WAR keeps it pipelined)
  stage B: V[k,e]  = Xv^T-tiles.T @ Wv       (staged to DRAM in an
           [k_p, et, k_o, e] layout so stage-D slab reads are contiguous)
  stage D per slot: S^T[k,q] = KT-tiles.T @ Q^T, exp+mask on S^T,
           den = ones.T @ expS (denominator replicated on all partitions),
           O^T[e,q] = V-slab-tiles.T @ expS^T, scaled by 1/den.
Output is O^T per core; the host reassembles [B,S,D].

Perf notes: ~296us HW time on 8 cores (TensorE ~88% busy, matmul p50
230ns at N=512).  DMA queues: bulk loads ride HWDGE (nc.sync), stores +
small loads ride SWDGE (nc.gpsimd) so PE load-waits never sit behind
result-dependent stores; walrus accepts only ONE sync-wait per
instruction, so _split_multi_waits() splits extras onto wait-only NoOps.
"""
import sys
import types

sys.path.insert(0, "/opt/trn_rl_repo")


def _install_ntff_hook():
    try:
        import antenv
    except ImportError:
        return

    if "antenv.axon_hooks" in sys.modules:
        return
    mod = types.ModuleType("antenv.axon_hooks")
    _h = [None]
    mod.set_axon_ntff_profile_hook = lambda h: _h.__setitem__(0, h)
    mod.get_axon_ntff_profile_hook = lambda: _h[0]
    sys.modules["antenv.axon_hooks"] = mod
    antenv.axon_hooks = mod
    try:
        from trn_agent_boot.trn_boot import _ntff_profile_via_ctypes

        mod.set_axon_ntff_profile_hook(
            _ntff_profile_via_ctypes("/opt/axon/libaxon_pjrt.so"))
    except Exception:
        pass


_install_ntff_hook()


import numpy as np
import concourse.bass as bass
import concourse.tile as tile
from concourse import mybir
from concourse.bass_utils import run_bass_kernel_spmd

P = 128
B, S, D = 4, 2048, 1024
N = 512                      # matmul moving free dim / queries per slot
NCORES = 8
K_SLOTS = (8, 16)            # k-tiles per phase slot (uniform across cores)
Q0S = {0: (0, 1536), 1: (512, 1024)}   # slot query starts per core parity
SCALE = float(1.0 / np.sqrt(np.float32(2048)))

f32 = mybir.dt.float32
bf16 = mybir.dt.bfloat16
fp8 = mybir.dt.float8e4
DR = mybir.MatmulPerfMode.DoubleRow
EXP = mybir.ActivationFunctionType.Exp
MULT = mybir.AluOpType.mult


def _split_multi_waits(nc):
    """Walrus allows one sync-wait per instruction; split extras onto
    wait-only NoOps inserted right before the offending instruction."""
    for f in nc.m.functions:
        for bb in f.blocks:
            insts = bb.instructions
            i = 0
            while i < len(insts):
                ins = insts[i]
                si = getattr(ins, "sync_info", None)
                if si and si.on_wait and len(si.on_wait) > 1:
                    waits = list(si.on_wait)
                    for j, w in enumerate(waits[:-1]):
                        nop = mybir.InstNoOp(
                            name=f"{ins.name}-waitsplit-{j}",
                            sync_info=mybir.SyncInfo(on_wait=[w], on_update=[]),
                            bass_nofuse=True,
                            engine=ins.engine, ins=[], outs=[])
                        insts.insert(i + j, nop)
                    i += len(waits) - 1
                    ins.sync_info = mybir.SyncInfo(
                        on_wait=[waits[-1]], on_update=list(si.on_update))
                i += 1


def build():
    nc = bass.Bass()
    # all host-side tensors are pre-arranged into SBUF layout [dp, do, cols]
    wq = nc.dram_tensor("wq", [P, 8, 8, P], fp8, kind="ExternalInput")
    wk = nc.dram_tensor("wk", [P, 8, D], fp8, kind="ExternalInput")
    wv = nc.dram_tensor("wv", [P, 8, D], bf16, kind="ExternalInput")
    wv8 = nc.dram_tensor("wv8", [P, 8, D], fp8, kind="ExternalInput")
    xqt = nc.dram_tensor("xqt", [P, 8, 1024], fp8, kind="ExternalInput")
    xkt = nc.dram_tensor("xkt", [P, 8, S], fp8, kind="ExternalInput")
    xvt = nc.dram_tensor("xvt", [P, 8, 1024], bf16, kind="ExternalInput")
    xvt8 = nc.dram_tensor("xvt8", [P, 8, 1024], fp8, kind="ExternalInput")
    thr = nc.dram_tensor("thr", [P, 2, 16], f32, kind="ExternalInput")
    iot = nc.dram_tensor("iota", [P, N], f32, kind="ExternalInput")
    out = nc.dram_tensor("out", [D, 1024], f32, kind="ExternalOutput")

    with tile.TileContext(nc) as tc:
        from contextlib import ExitStack
        with ExitStack() as ctx:
            kt_pool = ctx.enter_context(tc.tile_pool(name="ktp", bufs=1))
            xh_pool = ctx.enter_context(tc.tile_pool(name="xh", bufs=1))
            qt_pool = ctx.enter_context(tc.tile_pool(name="qtp", bufs=1))
            sm_pool = ctx.enter_context(tc.tile_pool(name="sm", bufs=1))
            psum = ctx.enter_context(
                tc.tile_pool(name="ps", bufs=8, space="PSUM"))
            dram = ctx.enter_context(
                tc.tile_pool(name="dram", bufs=1, space="DRAM"))

            v_dram = dram.tile([P, 8, 16, P], bf16)  # V: [k_p, et, k_o, e]
            v8_dram = dram.tile([P, 8, 16, P], fp8)  # fp8 copy for slot 1
            KT = kt_pool.tile([P, 8, S], fp8)        # K^T: [e_p, e_o, k]
            QT = qt_pool.tile([P, 8, 1024], fp8)     # Q^T: [e_p, e_o, q_loc]

            ones = sm_pool.tile([P, P], bf16)
            nc.gpsimd.memset(ones[:], 1.0)
            ones8 = sm_pool.tile([P, 2, P], fp8)
            nc.gpsimd.memset(ones8[:], 1.0)

            # wv tiles allocated up front, in a region disjoint from the
            # stage-A/C pools: a stage-B-scoped pool would reuse the region
            # those pools free, and the region-WAR blocks the wv loads (and
            # the whole sync queue behind them) until stage C drains.
            wv_pool = ctx.enter_context(tc.tile_pool(name="wvp", bufs=1))
            wv_sb = wv_pool.tile([P, 8, D], bf16, tag="wv", name="wv_sb")
            wv8_sb = wv_pool.tile([P, 8, D], fp8, tag="wv8", name="wv8_sb")

            ET_GROUPS = ((0, 2), (2, 4), (4, 6), (6, 8))

            # warm up the PE HAM clock while the first input strips stream in
            wps = psum.tile([P, N], f32, tag="ps", name="warmps")
            for i in range(24):
                nc.tensor.matmul(wps[:, 0:P], ones[:], ones[:],
                                 start=(i == 0), stop=(i == 23))

            def copy_alt(i, dst, src):
                if i % 2 == 0:
                    nc.vector.tensor_copy(dst, src)
                else:
                    nc.scalar.copy(dst, src)

            # w_sb is overwritten in place three times (wk -> wq -> wv);
            # range-based tracking gives per-row WAR deps, so each overwrite
            # streams in while later rows are still being consumed.
            with tc.tile_pool(name="wres", bufs=1) as wres:
                w_sb = wres.tile([P, 8, D], fp8, tag="w")

                # ---- Stage A: K^T[e,k] = sum_d Wk-tiles.T @ Xk^T[d,k] ----
                # loads ride 3 queues in d-pair strips matching DR t-pairs,
                # so descriptor generation (~0.7us/desc/queue) never starves
                # the PE.
                QS = (nc.sync, nc.scalar, nc.gpsimd)
                # double-buffered xk halves: every load below is dep-free, so
                # the queues stream back-to-back with no head-of-line WAR
                # blocking.
                xk_hs = [xh_pool.tile([P, 8, 1024], fp8, tag=f"xk{h}",
                                      name=f"xk{h}") for h in range(2)]
                for d in range(8):
                    QS[d % 3].dma_start(w_sb[:, d, :], wk[:, d, :])
                    QS[(d + 1) % 3].dma_start(xk_hs[0][:, d, :],
                                              xkt[:, d, 0:1024])
                for d in range(8):
                    QS[(d + 2) % 3].dma_start(xk_hs[1][:, d, :],
                                              xkt[:, d, 1024:2048])
                for half in range(2):
                    xk_h = xk_hs[half]
                    for g0, g1 in ET_GROUPS:
                        pss = {}
                        for et in range(g0, g1):
                            for kc in range(2):
                                pss[(et, kc)] = psum.tile(
                                    [P, N], f32, tag="ps",
                                    name=f"psa{half}_{et}_{kc}")
                        for t in range(4):
                            for et in range(g0, g1):
                                lhs = w_sb[:, 2 * t:2 * t + 2,
                                           et * P:(et + 1) * P]
                                for kc in range(2):
                                    nc.tensor.matmul(
                                        pss[(et, kc)][:], lhs,
                                        xk_h[:, 2 * t:2 * t + 2,
                                             kc * N:(kc + 1) * N],
                                        start=(t == 0), stop=(t == 3),
                                        perf_mode=DR)
                        for i, et in enumerate(range(g0, g1)):
                            for kc in range(2):
                                col = half * 1024 + kc * N
                                copy_alt(i + kc, KT[:, et, col:col + N],
                                         pss[(et, kc)][:])

                # ---- Stage C: Q^T[e,q] = sum_d Wq-tiles.T @ Xq^T[d,q] ----
                # wq gets its own tile (no WAR on w_sb) so its loads stream
                # during stage A; wq_sb[:, et, d*P:(d+1)*P] holds
                # Wq[d*P:(d+1)*P, et*P:(et+1)*P]
                wq_sb = wres.tile([P, 8, D], fp8, tag="wq", name="wq_sb")
                for et in range(8):
                    QS[et % 3].dma_start(wq_sb[:, et, :], wq[:, et, :, :])
                with tc.tile_pool(name="xqs", bufs=8) as xq_pool:
                    xqhs = {}
                    for qc in range(2):
                        for t in range(4):
                            xqh = xq_pool.tile([P, 2, N], fp8, tag="xq",
                                               name=f"xq{qc}_{t}")
                            QS[(qc * 4 + t) % 3].dma_start(
                                xqh[:],
                                xqt[:, 2 * t:2 * t + 2, qc * N:(qc + 1) * N])
                            xqhs[(qc, t)] = xqh
                    # 4 PSUM banks per et-group so consecutive groups pipeline
                    for qc in range(2):
                        for eg in range(2):
                            ets = range(4 * eg, 4 * eg + 4)
                            psq = {et: psum.tile([P, N], f32, tag="ps",
                                                 name=f"psq{qc}_{et}")
                                   for et in ets}
                            for t in range(4):
                                for et in ets:
                                    lhs = wq_sb[
                                        :, et, 2 * t * P:(2 * t + 2) * P
                                    ].rearrange("p (two f) -> p two f", two=2)
                                    nc.tensor.matmul(
                                        psq[et][:], lhs, xqhs[(qc, t)][:],
                                        start=(t == 0), stop=(t == 3),
                                        perf_mode=DR)
                            for et in ets:
                                copy_alt(et, QT[:, et, qc * N:(qc + 1) * N],
                                         psq[et][:])

            # ---- Stage B: V[k,e] = sum_d Xv^T-tiles.T @ Wv[d,e] ----
            iota_sb = sm_pool.tile([P, N], f32)
            nc.sync.dma_start(iota_sb[:], iot[:])
            thr_sb = sm_pool.tile([P, 2, 16], f32)
            nc.sync.dma_start(thr_sb[:], thr[:])
            # stage-D masks precomputed here: DVE is idle during stage B
            mk_pool = ctx.enter_context(tc.tile_pool(name="mk", bufs=1))
            masks = {}
            for s in range(2):
                mdt = bf16 if s == 0 else fp8
                for kt in range(K_SLOTS[s]):
                    if s == 1 and kt < 8:
                        continue
                    mk = mk_pool.tile([P, N], mdt, tag=f"mk{s}_{kt}",
                                      name=f"mk{s}_{kt}")
                    nc.vector.tensor_scalar(
                        out=mk[:], in0=iota_sb[:],
                        scalar1=thr_sb[:, s, kt:kt + 1], scalar2=None,
                        op0=mybir.AluOpType.is_ge)
                    masks[(s, kt)] = mk
            if True:  # keep indent level of the former wres scope
                with tc.tile_pool(name="vp", bufs=3) as v_pool:
                    for d in range(8):
                        QS[d % 3].dma_start(wv_sb[:, d, :], wv[:, d, :])
                    # half 0 (keys 0..1023, feeds the earliest queries) stays
                    # bf16; half 1 (keys 1024..2047, only ever attended with
                    # n_eff >= 1024) runs fp8 DoubleRow.
                    xv_h0 = xh_pool.tile([P, 8, 1024], bf16, tag="xv0",
                                         name="xv0")
                    xv_h1 = xh_pool.tile([P, 8, 1024], fp8, tag="xv1",
                                         name="xv1")
                    for d in range(8):
                        QS[(d + 1) % 3].dma_start(xv_h0[:, d, :],
                                                  xvt[:, d, :])
                    for d in range(8):
                        QS[(d + 2) % 3].dma_start(xv_h1[:, d, :],
                                                  xvt8[:, d, :])
                        QS[d % 3].dma_start(wv8_sb[:, d, :], wv8[:, d, :])
                    for half in range(2):
                        for g0, g1 in ((0, 2), (2, 4), (4, 6), (6, 8)):
                            ps2 = {}
                            for ktl in range(g0, g1):
                                for ec in range(2):
                                    ps2[(ktl, ec)] = psum.tile(
                                        [P, N], f32, tag="ps",
                                        name=f"psb{half}_{ktl}_{ec}")
                            if half == 0:
                                for d in range(8):
                                    for ktl in range(g0, g1):
                                        lhs = xv_h0[:, d,
                                                    ktl * P:(ktl + 1) * P]
                                        for ec in range(2):
                                            nc.tensor.matmul(
                                                ps2[(ktl, ec)][:], lhs,
                                                wv_sb[:, d,
                                                      ec * N:(ec + 1) * N],
                                                start=(d == 0), stop=(d == 7))
                            else:
                                for t in range(4):
                                    for ktl in range(g0, g1):
                                        lhs = xv_h1[:, 2 * t:2 * t + 2,
                                                    ktl * P:(ktl + 1) * P]
                                        for ec in range(2):
                                            nc.tensor.matmul(
                                                ps2[(ktl, ec)][:], lhs,
                                                wv8_sb[:, 2 * t:2 * t + 2,
                                                       ec * N:(ec + 1) * N],
                                                start=(t == 0), stop=(t == 3),
                                                perf_mode=DR)
                            for ktl in range(g0, g1):
                                ktg = half * 8 + ktl
                                for ec in range(2):
                                    vt8 = v_pool.tile([P, N], fp8,
                                                      tag="vst8", name="vt8")
                                    if half == 0:
                                        # slot 0 needs bf16 V; slot 1 reads
                                        # the fp8 copy (cast on idle DVE)
                                        vt = v_pool.tile([P, N], bf16,
                                                         tag="vst")
                                        nc.scalar.copy(vt[:],
                                                       ps2[(ktl, ec)][:])
                                        nc.vector.tensor_copy(vt8[:], vt[:])
                                        nc.scalar.dma_start(
                                            v_dram[:, ec * 4:(ec + 1) * 4,
                                                   ktg, :],
                                            vt[:].rearrange(
                                                "p (et e) -> p et e", e=P))
                                    else:
                                        nc.scalar.copy(vt8[:],
                                                       ps2[(ktl, ec)][:])
                                    nc.gpsimd.dma_start(
                                        v8_dram[:, ec * 4:(ec + 1) * 4,
                                                ktg, :],
                                        vt8[:].rearrange(
                                            "p (et e) -> p et e", e=P))

            # ---- Stage D: per phase slot: scores, softmax, O^T ----
            # masks precomputed on DVE (overlaps stage B); the per-kt apply
            # rides the otherwise-idle gpsimd engine.
            vin_pool = ctx.enter_context(tc.tile_pool(name="vin", bufs=6))
            out_pool = ctx.enter_context(tc.tile_pool(name="op", bufs=3))
            rd_pool = ctx.enter_context(tc.tile_pool(name="rd", bufs=2))
            v8_pool = ctx.enter_context(tc.tile_pool(name="v8", bufs=3))
            for s in range(2):
                K = K_SLOTS[s]
                # slot 0 holds the earliest queries -> bf16 attention weights
                # and V; slot 1 (n_eff >= 1024 keys) runs fp8 end-to-end.
                sdt = bf16 if s == 0 else fp8
                # scores^T -> exp -> causal/pad mask
                expS = xh_pool.tile([P, 16, N], sdt, tag="xh",
                                    name=f"expS{s}")
                for kt in range(K):
                    ps = psum.tile([P, N], f32, tag="ps", name=f"pss{s}_{kt}")
                    for g in range(4):
                        nc.tensor.matmul(
                            ps[:], KT[:, 2 * g:2 * g + 2, kt * P:(kt + 1) * P],
                            QT[:, 2 * g:2 * g + 2, s * N:(s + 1) * N],
                            start=(g == 0), stop=(g == 3), perf_mode=DR)
                    nc.scalar.activation(expS[:, kt, :], ps[:], EXP,
                                         scale=SCALE)
                    if (s, kt) in masks:
                        nc.vector.tensor_tensor(
                            out=expS[:, kt, :], in0=expS[:, kt, :],
                            in1=masks[(s, kt)][:], op=MULT)

                # denominator, replicated on all partitions
                dps = psum.tile([P, N], f32, tag="ps", name=f"dps{s}")
                if s == 0:
                    for kt in range(K):
                        nc.tensor.matmul(dps[:], ones[:], expS[:, kt, :],
                                         start=(kt == 0), stop=(kt == K - 1))
                else:
                    for i in range(K // 2):
                        nc.tensor.matmul(
                            dps[:], ones8[:], expS[:, 2 * i:2 * i + 2, :],
                            start=(i == 0), stop=(i == K // 2 - 1),
                            perf_mode=DR)
                rden = rd_pool.tile([P, N], f32)
                nc.vector.reciprocal(rden[:], dps[:])

                # O^T[e,q] with per-et V slabs streamed from DRAM
                for et in range(8):
                    po = psum.tile([P, N], f32, tag="ps", name=f"po{s}_{et}")
                    if s == 0:
                        slab = vin_pool.tile([P, 8, P], bf16, tag="vs",
                                             name=f"vs{s}_{et}")
                        nc.sync.dma_start(slab[:], v_dram[:, et, :K, :])
                        for kt in range(K):
                            nc.tensor.matmul(po[:], slab[:, kt, :],
                                             expS[:, kt, :],
                                             start=(kt == 0),
                                             stop=(kt == K - 1))
                    else:
                        slab8 = v8_pool.tile([P, 16, P], fp8, tag="v8",
                                             name=f"v8_{et}")
                        nc.sync.dma_start(slab8[:], v8_dram[:, et, :, :])
                        for i in range(K // 2):
                            nc.tensor.matmul(
                                po[:], slab8[:, 2 * i:2 * i + 2, :],
                                expS[:, 2 * i:2 * i + 2, :],
                                start=(i == 0), stop=(i == K // 2 - 1),
                                perf_mode=DR)
                    ot = out_pool.tile([P, N], f32)
                    nc.vector.tensor_tensor(out=ot[:], in0=po[:],
                                            in1=rden[:], op=MULT)
                    nc.gpsimd.dma_start(
                        out[et * P:(et + 1) * P, s * N:(s + 1) * N], ot[:])

    _split_multi_waits(nc)
    return nc


_NC_CACHE = None


def _get_nc():
    global _NC_CACHE
    if _NC_CACHE is None:
        _NC_CACHE = build()
    return _NC_CACHE


def _sbufize(a):
    """[rows(1024), cols] -> [dp(128), do(8), cols] contiguous."""
    r, c = a.shape
    return np.ascontiguousarray(a.reshape(8, P, c).transpose(1, 0, 2))


def _host_prep(inputs_for_keys, inputs_for_values, inputs_for_queries,
               weight_q, weight_k, weight_v):
    import ml_dtypes
    bf = ml_dtypes.bfloat16
    f8 = ml_dtypes.float8_e4m3
    f = lambda a, t: np.asarray(a, dtype=np.float32).astype(t)
    ik, iq = f(inputs_for_keys, f8), f(inputs_for_queries, f8)
    iv = f(inputs_for_values, bf)
    iv8 = f(inputs_for_values, f8)
    wq_t = f(weight_q, f8).reshape(8, P, 8, P)      # [d_o, d_p, e_o, e_p]
    wq = np.ascontiguousarray(wq_t.transpose(1, 2, 0, 3))  # [d_p, et, d_o, e]
    wk = _sbufize(f(weight_k, f8))
    wv = _sbufize(f(weight_v, bf))
    wv8 = _sbufize(f(weight_v, f8))

    iota = np.broadcast_to(np.arange(N, dtype=np.float32), (P, N)).copy()
    in_maps = []
    for c in range(NCORES):
        b, h = c // 2, c % 2
        q0s = Q0S[h]
        xq = np.concatenate([iq[b, q0:q0 + 512] for q0 in q0s], axis=0)
        x = np.arange(P, dtype=np.float32)
        thr = np.empty((P, 2, 16), np.float32)
        for s_, q0 in enumerate(q0s):
            for kt in range(16):
                thr[:, s_, kt] = kt * P + x - q0
        in_maps.append({
            "wq": wq, "wk": wk, "wv": wv, "wv8": wv8,
            "xqt": _sbufize(np.ascontiguousarray(xq.T)),
            "xkt": _sbufize(np.ascontiguousarray(ik[b].T)),
            "xvt": _sbufize(np.ascontiguousarray(iv[b, 0:1024].T)),
            "xvt8": _sbufize(np.ascontiguousarray(iv8[b, 1024:2048].T)),
            "thr": thr, "iota": iota,
        })
    return in_maps


def _assemble(results):
    out = np.empty((B, S, D), np.float32)
    for c in range(NCORES):
        b, h = c // 2, c % 2
        oc = results[c]["out"].T        # [q_local, e]
        for s_, q0 in enumerate(Q0S[h]):
            out[b, q0:q0 + 512] = oc[s_ * 512:(s_ + 1) * 512]
    return out


def kernel(**inputs) -> np.ndarray:
    nc = _get_nc()
    in_maps = _host_prep(**inputs)
    res = run_bass_kernel_spmd(nc, in_maps, list(range(NCORES)))
    return _assemble(res.results)


def kernel_profiled(**inputs):
    """Like kernel() but also returns (output, exec_time_ns, results)."""
    nc = _get_nc()
    in_maps = _host_prep(**inputs)
    res = run_bass_kernel_spmd(nc, in_maps, list(range(NCORES)), trace=True)
    return _assemble(res.results), res.exec_time_ns, res

